# revision 1
# baseline (speedup 1.0000x reference)
"""ConvDownsample2D (StyleGAN2 FIR blur + strided conv) for 8 Trainium2 cores.

Sharding: data-parallel over batch, 1 image per NeuronCore.

v2 design — minimize instruction count (~1.3k vs 4.1k) and keep engines
balanced:
  * x arrives host-relaid CHANNEL-MAJOR [C, H, W] fp16 -> big contiguous DMAs
    and no on-device transpose at all.
  * Separable blur runs as a ping-pong cascade of large DVE adds over row
    bands ([1,3,3,1] = [1,1]^3 -> 3 adds per direction).
  * Conv runs WEIGHTS-STATIONARY: psum[oc128, 4 rows, 128 cols] accumulates
    9 taps of matmul(lhsT=w_tap[C,oc128], rhs=z[C, rows::2, cols::2]) with
    N=512 moving columns (the PE column floor: 576 matmuls/image).
  * Bias is folded into the PSUM evacuation on the scalar engine
    (activation Identity + per-partition bias).
  * Output is written channel-major [OC, OH, OW] and transposed on host.
"""
import sys

if "/opt/trn_rl_repo" not in sys.path:
    sys.path.insert(0, "/opt/trn_rl_repo")

import numpy as np

import concourse.bass as bass
import concourse.tile as tile
from concourse import bacc, mybir
from concourse.bass_utils import run_bass_kernel_spmd

F16 = mybir.dt.float16
F32 = mybir.dt.float32
F32R = mybir.dt.float32r

# "f32r": conv matmuls in float32r — self-loading (no InstLdweights), 1
# instruction per matmul, ~3.6x the per-matmul cost of fp16.
# "f16": conv matmuls in fp16 — 2 instructions per matmul (Ld+MM), fastest.
CONV_DTYPE = "f32r"

N_CORES = 8
C = 128
H = W = 256
OC = 256
OH = OW = 128
PITCH = 264          # fp16 row pitch; borders [0:4)+[260:264) are zeroed
XO = 4               # x col j at buffer col j+XO
NMAX = 36            # max band tile rows (2*16+4)

# out-row bands: small first/last band shortens pipeline fill/drain
BANDS = [(0, 4), (4, 8)] + [(12 + 16 * i, 16) for i in range(7)] + [(124, 4)]
assert BANDS[-1][0] + BANDS[-1][1] == OH

# fraction of each blur pass's rows offloaded to the (otherwise idle)
# GPSIMD engine; ~4.8x slower per element than DVE's 2x mode, so keep small
GPF = 0.20


def _build_bass(mode, repeat=1):
    nc = bacc.Bacc("TRN2", target_bir_lowering=False, debug=False)

    WDT = F32R if CONV_DTYPE == "f32r" else F16
    xc = nc.dram_tensor("xc", [C, H, W], F16, kind="ExternalInput").ap()
    w9 = nc.dram_tensor("w9", [C, 9, OC], WDT, kind="ExternalInput").ap()
    bias2 = nc.dram_tensor("bias2", [C, 2], F32, kind="ExternalInput").ap()
    if mode == "general":
        # kh[c,4] then kv[c,4] tap coefficients (broadcast per partition)
        kco = nc.dram_tensor("kco", [C, 8], F32, kind="ExternalInput").ap()
    out = nc.dram_tensor("out", [OC, OH, OW], F32, kind="ExternalOutput").ap()

    with tile.TileContext(nc) as tc:
        with (
            tc.tile_pool(name="const", bufs=1) as cpool,
            tc.tile_pool(name="apool", bufs=3) as apool,
            tc.tile_pool(name="bpool", bufs=2) as bpool,
            tc.tile_pool(name="zpool", bufs=2) as zpool,
            tc.tile_pool(name="osb", bufs=1) as opool,
            tc.tile_pool(name="ps", bufs=1, space=bass.MemorySpace.PSUM) as ppool,
        ):
            w_sb = cpool.tile([C, 9, OC], WDT)
            bias_sb = cpool.tile([C, 2], F32)
            if mode == "general":
                # needed by the very first blur op -> load up front
                kco_sb = cpool.tile([C, 8], F32, name="kco_sb")
                nc.sync.dma_start(kco_sb[:], kco[:])
            else:
                kco_sb = None

            def load_consts():
                # deferred so band-0/1 input DMAs lead the SP queue
                nc.sync.dma_start(w_sb[:], w9[:])
                nc.sync.dma_start(bias_sb[:], bias2[:])

            def blur_band(k):
                """Load + blur band k. Returns tile holding z:
                row t = y row (2*p0 + t), col j' = y col (j'-2) ... i.e.
                conv reads z rows (2r'+a), buffer cols (2s+b+2)."""
                p0, R = BANDS[k]
                n = 2 * R + 4
                lo = 2 * p0 - 2
                s_lo = max(0, -lo)
                s_hi = min(n, H - lo)

                A = apool.tile([C, NMAX, PITCH], F16, tag="A")
                B = bpool.tile([C, NMAX, PITCH], F16, tag="B")
                nc.sync.dma_start(
                    A[:, s_lo:s_hi, XO:XO + W],
                    xc[:, lo + s_lo:lo + s_hi, :],
                )
                # tiny border zeroes on DVE itself: they sit directly before
                # H1 in its FIFO, and their narrow column ranges don't
                # overlap the DMA region (a merged strided AP would span the
                # full row and serialize behind the input DMA)
                nc.vector.memset(A[:, 0:n, 0:XO], 0.0)
                nc.vector.memset(A[:, 0:n, XO + W:PITCH], 0.0)
                if s_lo > 0:
                    nc.vector.memset(A[:, 0:s_lo, XO:XO + W], 0.0)
                if s_hi < n:
                    nc.vector.memset(A[:, s_hi:n, XO:XO + W], 0.0)

                def finish(Zt):
                    return Zt

                def add(dst, nr, s0, rsh, c0, c1, csh):
                    """dst[:, 0:nr, c0:c1] = s0[:, rsh:rsh+nr, c0+csh:c1+csh]
                                           + s0[:, 0:nr, c0:c1] split DVE/GP."""
                    S = nr - max(1, int(round(GPF * nr))) if (GPF > 0 and R >= 8) else nr
                    nc.vector.tensor_add(
                        dst[:, 0:S, c0:c1],
                        s0[:, 0:S, c0:c1],
                        s0[:, rsh:rsh + S, c0 + csh:c1 + csh])
                    if S < nr:
                        nc.gpsimd.tensor_add(
                            dst[:, S:nr, c0:c1],
                            s0[:, S:nr, c0:c1],
                            s0[:, rsh + S:rsh + nr, c0 + csh:c1 + csh])

                if mode == "b1331":
                    # H: 3 shift-1 col adds, A->B->A->B
                    # u1 needed for y-cols <= 256 -> j in [-2,256] -> cols [2,261)
                    add(B, n, A, 0, 2, 261, 1)
                    add(A, n, B, 0, 2, 260, 1)
                    add(B, n, A, 0, 2, 259, 1)
                    # V: 3 shift-1 row adds, B->A->B->A
                    add(A, n - 1, B, 1, 2, 259, 0)
                    add(B, n - 2, A, 1, 2, 259, 0)
                    add(A, n - 3, B, 1, 2, 259, 0)
                    return finish(A)
                elif mode == "b1111":
                    # [1,1,1,1] = [1,1] * [1,0,1]: 2 adds per direction
                    add(B, n, A, 0, 2, 261, 1)
                    add(A, n, B, 0, 2, 259, 2)
                    add(B, n - 1, A, 1, 2, 259, 0)
                    add(A, n - 3, B, 2, 2, 259, 0)
                    return finish(A)
                else:
                    # general 4-tap: acc = sum_u k[u] * shift_u(x), per direction
                    stt = nc.vector.scalar_tensor_tensor
                    mul = mybir.AluOpType.mult
                    addop = mybir.AluOpType.add
                    nc.vector.tensor_scalar(
                        B[:, 0:n, 2:259], A[:, 0:n, 2:259], kco_sb[:, 0:1],
                        None, mul)
                    for u in range(1, 4):
                        stt(B[:, 0:n, 2:259], A[:, 0:n, 2 + u:259 + u],
                            kco_sb[:, u:u + 1], B[:, 0:n, 2:259], mul, addop)
                    nc.vector.tensor_scalar(
                        A[:, 0:n - 3, 2:259], B[:, 0:n - 3, 2:259], kco_sb[:, 4:5],
                        None, mul)
                    for u in range(1, 4):
                        stt(A[:, 0:n - 3, 2:259], B[:, u:n - 3 + u, 2:259],
                            kco_sb[:, 4 + u:5 + u], A[:, 0:n - 3, 2:259], mul, addop)
                    return finish(A)

            def finish_band(k, Zt):
                """Emit the f16->f32r convert for band k's blurred tile."""
                if CONV_DTYPE != "f32r":
                    return Zt, 2
                _, R = BANDS[k]
                n = 2 * R + 4
                # NMAX-2 rows: conv slices end at 8*(R//4-1)+a+8 <= 34
                # (last actually-read row is 32). Split so the first conv
                # chunk (reads rows <=16) can start after part a.
                Zr = zpool.tile([C, NMAX - 2, 258], F32R, tag="Zr")
                nz = n - 3
                if nz > 18:
                    nc.scalar.copy(Zr[:, 0:18, 0:257], Zt[:, 0:18, 2:259])
                    nc.scalar.copy(Zr[:, 18:nz, 0:257], Zt[:, 18:nz, 2:259])
                else:
                    nc.scalar.copy(Zr[:, 0:nz, 0:257], Zt[:, 0:nz, 2:259])
                return Zr, 0

            # out DRAM [OC=2*128, OH, OW] viewed per-partition as 2 oc chunks
            out4 = out.rearrange("(g p) r q -> p g r q", g=2)

            def conv_band(k, ZC, mid=None):
                Z, co = ZC
                p0, R = BANDS[k]
                assert R in (4, 8, 16), "psum chunking assumes 1/2/4 groups"
                ngrp = R // 4
                ot = opool.tile([C, 2, 16, OW], F32, tag="o")
                for half in range(2):
                    if half == 1 and mid is not None:
                        # emit the NEXT band's convert between the halves so
                        # it overlaps this band's second-half matmuls
                        mid()
                    # one 4-bank psum tile per half; its evac runs during the
                    # other half's matmuls, so the WAR on the next band's
                    # matmuls is released in time
                    ps = ppool.tile([C, 16, OW], F32, tag=f"ps{half}")
                    for t in range(9):
                        a, b = divmod(t, 3)
                        lhsT = w_sb[:, t, 128 * half:128 * (half + 1)]
                        for g in range(ngrp):
                            nc.tensor.matmul(
                                ps[:, 4 * g:4 * g + 4, :],
                                lhsT,
                                Z[:, 8 * g + a:8 * g + a + 8:2,
                                  b + co:b + co + 256:2],
                                start=(t == 0), stop=(t == 8),
                                skip_group_check=True,
                            )
                    nc.scalar.activation(
                        ot[:, half, 0:R, :], ps[:, 0:R, :],
                        mybir.ActivationFunctionType.Identity,
                        bias=bias_sb[:, half:half + 1],
                    )
                nc.scalar.dma_start(
                    out4[:, :, p0:p0 + R, :], ot[:, :, 0:R, :],
                )

            for rep in range(repeat):
                # conv(k-1) emitted BEFORE blur(k)'s DVE chain completes;
                # band k's convert is emitted between conv(k-1)'s two halves
                # so ACT runs [evac(k-1,h0), cvt(k), evac(k-1,h1)] and the
                # PE never waits a full convert between bands
                raw = blur_band(0)
                if rep == 0:
                    load_consts()
                prev_zc = finish_band(0, raw)
                for k in range(1, len(BANDS)):
                    raw = blur_band(k)
                    holder = []
                    conv_band(k - 1, prev_zc,
                              mid=lambda kk=k, rr=raw: holder.append(
                                  finish_band(kk, rr)))
                    prev_zc = holder[0]
                conv_band(len(BANDS) - 1, prev_zc)

    nc.compile()
    return nc


_NC = {}


def _get_nc(mode="b1331", repeat=1):
    key = (mode, repeat)
    if key not in _NC:
        _NC[key] = _build_bass(mode, repeat)
    return _NC[key]


def _blur_mode(bk):
    k8 = bk / bk.sum() * 8.0
    if np.allclose(k8, [1.0, 3.0, 3.0, 1.0], rtol=1e-6, atol=1e-7):
        return "b1331"
    k4 = bk / bk.sum() * 4.0
    if np.allclose(k4, [1.0, 1.0, 1.0, 1.0], rtol=1e-6, atol=1e-7):
        return "b1111"
    return "general"


def _prepare_in_maps(x, conv_w, conv_b, blur_kernel):
    x = np.asarray(x, dtype=np.float32)
    conv_w = np.asarray(conv_w, dtype=np.float32)
    conv_b = np.asarray(conv_b, dtype=np.float32)
    bk = np.asarray(blur_kernel, dtype=np.float32)

    mode = _blur_mode(bk)
    if mode in ("b1331", "b1111"):
        # device cascade computes the unnormalized integer-tap blur;
        # fold the 2D normalization into the conv weights
        wscale = 1.0 / (bk.sum() ** 2)
    else:
        wscale = 1.0  # normalized taps shipped via kco

    wdt = np.float32 if CONV_DTYPE == "f32r" else np.float16
    w9 = np.ascontiguousarray(
        (conv_w * wscale).reshape(9, C, OC).transpose(1, 0, 2).astype(wdt)
    )
    bias2 = np.ascontiguousarray(conv_b.reshape(2, 128).T.astype(np.float32))

    base = {"w9": w9, "bias2": bias2}
    if mode == "general":
        k1 = (bk / bk.sum()).astype(np.float32)
        kco = np.broadcast_to(
            np.concatenate([k1, k1])[None, :], (C, 8)
        ).astype(np.float32)
        base["kco"] = np.ascontiguousarray(kco)

    in_maps = []
    for i in range(N_CORES):
        im = dict(base)
        im["xc"] = np.ascontiguousarray(x[i].transpose(2, 0, 1).astype(np.float16))
        in_maps.append(im)
    return mode, in_maps


def _run(mode, in_maps, **kwargs):
    nc = _get_nc(mode)
    return run_bass_kernel_spmd(nc, in_maps, core_ids=list(range(N_CORES)), **kwargs)


def kernel(x, conv_w, conv_b, blur_kernel):
    mode, in_maps = _prepare_in_maps(x, conv_w, conv_b, blur_kernel)
    res = _run(mode, in_maps)
    # device output is channel-major [OC, OH, OW] -> NHWC
    return np.stack(
        [res.results[i]["out"].transpose(1, 2, 0) for i in range(N_CORES)], axis=0
    )

